# revision 7
# baseline (speedup 1.0000x reference)
"""Embedding lookup (gather) kernel for Trainium2, 8 NeuronCores.

Problem: out[i] = table[value_tensors[i]] for 212992 indices into a
[1M, 128] f32 table, reshaped to [8192, 26, 128]. (row_offsets is
arange, so the CSR segment-sum is the identity; a host-side fallback
handles the general case.)

Sharding: model-parallel by table row (range partition). The table is
split into 32 range bins of 31250 rows; core c owns bins 4c..4c+3.
The host dedupes and routes each lookup index to its owning bin, each
core gathers its rows on-device with the SWDGE dma_gather instruction,
and the host scatters the gathered rows back to the original positions
(the "all-to-all" of HugeCTR's localized embedding, at unshard time).

Perf model (from ntff traces of the 83-88us baselines):
  - The binding resource is Q7 DESCRIPTOR EMISSION: ~7ns/desc per call,
    with at most ENG_EXEC_QUEUE_DEPTH[Pool]=4 calls concurrent. So the
    gather phase >= total_descs * 7/4 ns. Minimize descriptors at zero
    garbage: one desc per unique row, plus a 512B-desc class for
    ADJACENT unique pairs (~16% of rows pair; 20.1k descs/core vs
    23.9k plain). Window classes beyond span 2 read+write garbage that
    the 16 DMA engines (the secondary bound: ~9ns+bytes/27GBps per
    desc, serial per engine) have to carry twice.
  - single_packet=True (<=57 descs/ring/call, CH=896) is load-bearing:
    single_packet=False degrades to per-descriptor ring packets (~32
    vs ~14-18ns/desc at the DMA engine).
  - PREPARE_ONLY + batched lagged triggers: a gen_mode=0 call retires
    only when its DMA completes, so 4-deep rounds ran at call-lifetime
    cadence. Preps retire at emission end; triggers are batched 4 at a
    time, lagged 8 calls, so their prep-sem waits resolve while the
    pipeline is still draining earlier preps (sem-prop hidden). Ring
    capacity check: 2 calls x 57 descs = 114 <= 128 slots/ring/queue.
  - Warm idx from a DVE memset (no input dependency); the warmup call
    absorbs the ~10us cold Q7/ucode library init that follows
    load_library before the first dma_gather can execute.
  - fp16 table/output halve both HBM directions (rel err 2^-11 vs the
    2e-2 gate); per-bin output writes overlap the gathers on the two
    HWDGE rings; the last bin's final (tiny) chunk gets its own sem so
    the tail write waits only on it.

dma_gather layout (probed on HW): indices are int16, wrapped over 16
partitions (ordinal i reads idx[i % 16, i // 16]) and replicated to all
8 Q7-core partition groups; gathered ordinal i lands at
dst[i % 128, i // 128]; negative idxs at the end generate no
descriptors, but each call keeps >= 16 leading non-negative idxs so all
16 engine rings still fire their completion-semaphore descriptor.
"""

import time

import numpy as np

VOCAB = 1_000_000
BATCH = 8192
SLOTS = 26
VEC = 128
NCORES = 8
NSUB = 4  # bins per core; int16 gather idx needs rows <= 32767
RSUB = VOCAB // (NCORES * NSUB)  # 31250 rows per bin
SHARD = RSUB * NSUB  # 125000 rows per core
P = 128
CH = 896  # idxs per call: 56 data + 1 sem desc per ring, <= 64 ceiling
NCLS = 2  # class 0: single rows (256B descs); class 1: adjacent pairs (512B)
ROWS_PER = [1, 2]
TRIG_LAG = 8  # preps to run ahead of triggers (2 calls/queue in ring)

LAST_RUN = None  # BassKernelResults of the most recent device run (for test.py)


def _chunks_of(N: int):
    out = []
    o = 0
    while o < N:
        out.append((o, min(CH, N - o)))
        o += CH
    return out


def _build_program(NCL: list, chunks: list):
    """One SPMD program for all 8 cores. NCL[c] = padded idx slots for
    class c per bin (multiples of 128, identical across cores/bins).

    Per core:
      shard [SHARD, VEC] fp16   - this core's 4 bins, concatenated
      idx   [P, ICOLS] i16      - [bin0 c0,c1][bin1 c0,c1]...
      cnt   [1, NCALL] i32      - per-gather-call runtime num_idxs
      out   [P, NSUB*W] fp16    - W = NCL[0] + 2*NCL[1] cols per bin
    """
    import bass_rust
    import concourse.bacc as bacc
    from concourse import mybir
    from concourse.library_config import mlp

    ncalls_bin = sum(len(ch) for ch in chunks)
    icols_bin = sum(NCL) // 16
    ccols = [NCL[c] * ROWS_PER[c] for c in range(NCLS)]
    roff = [0, ccols[0]]
    W = sum(ccols)
    ICOLS = NSUB * icols_bin
    NCALL = NSUB * ncalls_bin

    nc = bacc.Bacc("TRN2", num_swdge_queues=4)
    shard = nc.declare_dram_parameter(
        "shard", [SHARD, VEC], mybir.dt.float16, isOutput=False
    )
    idx = nc.declare_dram_parameter("idx", [P, ICOLS], mybir.dt.int16, isOutput=False)
    cnt = nc.declare_dram_parameter("cnt", [1, NCALL], mybir.dt.int32, isOutput=False)
    out = nc.declare_dram_parameter(
        "out", [P, NSUB * W], mybir.dt.float16, isOutput=True
    )

    sem_in = nc.alloc_semaphore("sem_in")
    sem_warm = nc.alloc_semaphore("sem_warm")
    sem_wi = nc.alloc_semaphore("sem_wi")
    sem_prep = nc.alloc_semaphore("sem_prep")
    # per-bin gather sems; the last bin's final chunk gets its own sem
    # so the tail write waits only on it.
    sem_g = [nc.alloc_semaphore(f"sem_g{s}") for s in range(NSUB + 1)]
    sem_out = nc.alloc_semaphore()

    idx_sb = nc.alloc_sbuf_tensor("idx_sb", [P, ICOLS], mybir.dt.int16).ap()
    warm_idx = nc.alloc_sbuf_tensor("warm_idx", [P, 8], mybir.dt.int16).ap()
    cnt_sb = nc.alloc_sbuf_tensor("cnt_sb", [1, NCALL], mybir.dt.int32).ap()
    warm_out = nc.alloc_sbuf_tensor("warm_out", [P, 1, VEC], mybir.dt.float16).ap()
    g_buf = nc.alloc_sbuf_tensor("g", [P, NSUB * W], mybir.dt.float16).ap()

    nc.gpsimd.load_library(mlp)
    nc.vector.memset(warm_idx, 0).then_inc(sem_wi, 1)
    nc.sync.dma_start(out=cnt_sb[:], in_=cnt[:, :]).then_inc(sem_in, 16)
    for s in range(NSUB):
        a, b = s * icols_bin, (s + 1) * icols_bin
        nc.sync.dma_start(out=idx_sb[:, a:b], in_=idx[:, a:b]).then_inc(sem_in, 16)

    warm_reg = nc.gpsimd.to_reg(128)
    cregs = [nc.gpsimd.alloc_register(name=f"creg{t}") for t in range(NCALL)]

    nc.gpsimd.wait_ge(sem_wi, 1)
    nc.gpsimd.dma_gather(
        warm_out[:, :, :],
        shard[0:RSUB, :],
        warm_idx,
        128,
        warm_reg,
        VEC,
        queue_num=0,
    ).then_inc(sem_warm, 16)

    # Batched loads, <= 24 regs each (52-wide measured failing to lower).
    nc.gpsimd.wait_ge(sem_in, 16)
    for i in range(0, NCALL, 24):
        j = min(i + 24, NCALL)
        nc.gpsimd.reg_load(cregs[i:j], cnt_sb[0:1, i:j])

    qn = 1  # warmup used q0
    t = 0
    trig_q = []  # queue of call t (FIFO); triggers batched 4, lagged TRIG_LAG
    ntrig = 0

    def fire_triggers(upto):
        nonlocal ntrig
        if upto > ntrig:
            nc.gpsimd.wait_ge(sem_prep, upto)
            while ntrig < upto:
                nc.gpsimd.trigger_dma(count=1, queue_num=trig_q[ntrig])
                ntrig += 1

    for s in range(NSUB):
        nc.gpsimd.wait_ge(sem_in, 16 * (s + 2))
        for c in range(NCLS):
            L = ROWS_PER[c]
            ibase = s * icols_bin + sum(NCL[:c]) // 16
            if L == 1:
                view = shard[s * RSUB : (s + 1) * RSUB, :]
            else:
                view = shard[s * RSUB : s * RSUB + (RSUB - L + 1), :].copy()
                view.ap = bass_rust.VecI64Pair([[VEC, RSUB - L + 1], [1, L * VEC]])
            for o, sz in chunks[c]:
                if s < NSUB - 1 or not (c == NCLS - 1 and o == chunks[c][-1][0]):
                    sem = sem_g[s]
                else:
                    sem = sem_g[NSUB]
                c0 = s * W + roff[c] + (o // 128) * L * VEC
                c1 = s * W + roff[c] + ((o + sz) // 128) * L * VEC
                dst = g_buf[:, c0:c1].rearrange("p (k e) -> p k e", e=L * VEC)
                nc.gpsimd.dma_gather(
                    dst,
                    view,
                    idx_sb[:, ibase + o // 16 : ibase + (o + sz) // 16],
                    sz,
                    cregs[t],
                    L * VEC,
                    elem_step=VEC if L > 1 else None,
                    prepare_only=True,
                    sem=sem,
                    queue_num=qn % 4,
                ).then_inc(sem_prep, 1)
                trig_q.append(qn % 4)
                qn += 1
                t += 1
                if t % 4 == 0 and t >= TRIG_LAG:
                    fire_triggers(t - (TRIG_LAG - 4))
    assert t == NCALL
    fire_triggers(NCALL)

    # Whole-bin writes alternating between the two HWDGE rings
    # (Sync/Scalar); the last bin is split [head | final chunk] so the
    # tail write waits only on the final (tiny) gather call.
    last_o = chunks[NCLS - 1][-1][0]
    split = roff[NCLS - 1] + (last_o // 128) * ROWS_PER[NCLS - 1] * VEC
    writes = []  # (engine_idx, sem, need, col0, col1)
    for s in range(NSUB - 1):
        writes.append((s % 2, sem_g[s], 16 * ncalls_bin, s * W, (s + 1) * W))
    s = NSUB - 1
    writes.append((s % 2, sem_g[s], 16 * (ncalls_bin - 1), s * W, s * W + split))
    writes.append((NSUB % 2, sem_g[NSUB], 16, s * W + split, NSUB * W))
    for ei, sem, need, c0, c1 in writes:
        eng = nc.sync if ei == 0 else nc.scalar
        eng.wait_ge(sem, need)
        eng.dma_start(out=out[:, c0:c1], in_=g_buf[:, c0:c1]).then_inc(sem_out, 16)
    nc.sync.wait_ge(sem_out, 16 * len(writes))
    nc.sync.wait_ge(sem_warm, 16)
    nc.finalize()
    return nc


def _wrap_cols(vals: np.ndarray, N: int, ecount: int) -> np.ndarray:
    """int16 idx block [16, N//16]: element i at [i%16, i//16]; slots
    [len(vals), ecount) hold 0 (valid row, gathered then ignored), slots
    [ecount, N) hold -1 (skipped by the ucode)."""
    li = np.full(N, -1, np.int16)
    li[:ecount] = 0
    li[: len(vals)] = vals.astype(np.int16)
    return li.reshape(N // 16, 16).T


def _split_pairs(rows: np.ndarray):
    """Greedy adjacent pairing of sorted unique rows: returns
    (singles, pair_starts)."""
    n = len(rows)
    if n == 0:
        return rows, rows
    run_start = np.concatenate(([True], np.diff(rows) != 1))
    run_id = np.cumsum(run_start) - 1
    starts = np.flatnonzero(run_start)
    lens = np.diff(np.append(starts, n))
    pos = np.arange(n) - starts[run_id]
    paired = 2 * (lens[run_id] // 2)
    is_pair_start = (pos % 2 == 0) & (pos < paired)
    is_single = pos >= paired
    return rows[is_single], rows[is_pair_start]


def _gather_on_device(table_f16: np.ndarray, uniq: np.ndarray) -> np.ndarray:
    """emb[i] = table[uniq[i]] (fp16) computed on 8 NeuronCores."""
    global LAST_RUN
    from concourse.bass_utils import run_bass_kernel_spmd

    total = uniq.shape[0]
    nbins = NCORES * NSUB
    bin_id = (uniq // RSUB).astype(np.int32)
    local = (uniq - bin_id.astype(np.int64) * RSUB).astype(np.int32)
    counts = np.bincount(bin_id, minlength=nbins)
    assert counts.sum() == total
    bin_start = np.concatenate(([0], np.cumsum(counts)))

    dec = []  # dec[b] = (singles, pair_starts)
    ncls_max = [0, 0]
    for b in range(nbins):
        sgl, prs = _split_pairs(local[bin_start[b] : bin_start[b + 1]])
        dec.append((sgl, prs))
        ncls_max[0] = max(ncls_max[0], len(sgl))
        ncls_max[1] = max(ncls_max[1], len(prs))
    NCL = [max(P, ((m + P - 1) // P) * P) for m in ncls_max]
    chunks = [_chunks_of(NCL[c]) for c in range(NCLS)]
    ncalls_bin = sum(len(ch) for ch in chunks)
    icols_bin = sum(NCL) // 16
    ccols = [NCL[c] * ROWS_PER[c] for c in range(NCLS)]
    roff = [0, ccols[0]]
    W = sum(ccols)

    in_maps = []
    for core in range(NCORES):
        blocks = []
        cvals = []
        for s in range(NSUB):
            b = core * NSUB + s
            for c in range(NCLS):
                vals = dec[b][c]
                n = len(vals)
                o_last = chunks[c][-1][0]
                ecount = max(n, o_last + 16)
                blocks.append(_wrap_cols(vals, NCL[c], ecount))
                for o, sz in chunks[c]:
                    cvals.append(min(max(ecount - o, 0), sz))
        in_maps.append(
            {
                "shard": np.ascontiguousarray(
                    table_f16[core * SHARD : (core + 1) * SHARD]
                ),
                "idx": np.ascontiguousarray(
                    np.tile(np.concatenate(blocks, axis=1), (8, 1))
                ),
                "cnt": np.array([cvals], np.int32),
            }
        )

    # The shared device occasionally wedges transiently
    # (NRT_EXEC_UNIT_UNRECOVERABLE / profile-stop rc=-1); a fresh attempt
    # after a short pause recovers it.
    for attempt in range(3):
        try:
            nc = _build_program(NCL, chunks)
            LAST_RUN = run_bass_kernel_spmd(nc, in_maps, list(range(NCORES)))
            break
        except Exception:
            if attempt == 2:
                raise
            time.sleep(10)
    res = LAST_RUN.results

    emb = np.empty((total, VEC), np.float16)
    for core in range(NCORES):
        o = np.asarray(res[core]["out"])
        for s in range(NSUB):
            b = core * NSUB + s
            sgl, prs = dec[b]
            reg = o[:, s * W : (s + 1) * W]
            bs = bin_start[b]
            loc = local[bs : bin_start[b + 1]]
            # class 0: singles; ordinal i at [i%128, (i//128)*VEC + :]
            if len(sgl):
                r0 = reg[:, roff[0] : roff[0] + ccols[0]]
                rows = (
                    r0.reshape(P, NCL[0] // 128, VEC).transpose(1, 0, 2).reshape(-1, VEC)
                )
                pos = np.searchsorted(loc, sgl)
                emb[bs + pos] = rows[: len(sgl)]
            # class 1: pairs; ordinal j covers rows (p_j, p_j + 1)
            if len(prs):
                r1 = reg[:, roff[1] : roff[1] + ccols[1]]
                pairs = (
                    r1.reshape(P, NCL[1] // 128, 2 * VEC)
                    .transpose(1, 0, 2)
                    .reshape(-1, 2, VEC)[: len(prs)]
                )
                pos0 = np.searchsorted(loc, prs)
                emb[bs + pos0] = pairs[:, 0]
                emb[bs + pos0 + 1] = pairs[:, 1]
    return emb


def kernel(table, row_offsets, value_tensors, nnz_array=None, output_shape=None):
    table = np.asarray(table, dtype=np.float32)
    assert table.shape == (VOCAB, VEC)
    v = np.asarray(value_tensors).astype(np.int64).ravel()
    total = v.shape[0]

    table_f16 = table.astype(np.float16)
    uniq, inverse = np.unique(v, return_inverse=True)
    emb_u = _gather_on_device(table_f16, uniq)
    emb = emb_u[inverse].astype(np.float32)

    n_rows = BATCH * SLOTS
    ro = np.asarray(row_offsets).astype(np.int64).ravel()
    if total == n_rows and np.array_equal(ro, np.arange(total + 1)):
        return emb.reshape(BATCH, SLOTS, VEC)
    # General CSR fallback (never hit with the reference's arange offsets):
    # sum-combine values per segment on the host.
    seg = np.searchsorted(ro, np.arange(total), side="right") - 1
    combined = np.zeros((n_rows, VEC), np.float32)
    np.add.at(combined, seg, emb)
    return combined.reshape(BATCH, SLOTS, VEC)


# revision 11
# speedup vs baseline: 1.0316x; 1.0316x over previous
"""Embedding lookup (gather) kernel for Trainium2, 8 NeuronCores.

Problem: out[i] = table[value_tensors[i]] for 212992 indices into a
[1M, 128] f32 table, reshaped to [8192, 26, 128]. (row_offsets is
arange, so the CSR segment-sum is the identity; a host-side fallback
handles the general case.)

Sharding: model-parallel by table row (range partition). The table is
split into 32 range bins of 31250 rows; core c owns bins 4c..4c+3.
The host dedupes and routes each lookup index to its owning bin, each
core gathers its rows on-device with the SWDGE dma_gather instruction,
and the host scatters the gathered rows back to the original positions
(the "all-to-all" of HugeCTR's localized embedding, at unshard time).

Perf model (from ntff traces of the 83-88us baselines):
  - The binding resource is Q7 DESCRIPTOR EMISSION: ~7ns/desc per call,
    with at most ENG_EXEC_QUEUE_DEPTH[Pool]=4 calls concurrent. So the
    gather phase >= total_descs * 7/4 ns. Minimize descriptors at zero
    garbage: one desc per unique row, plus a 512B-desc class for
    ADJACENT unique pairs (~16% of rows pair; 20.1k descs/core vs
    23.9k plain). Window classes beyond span 2 read+write garbage that
    the 16 DMA engines (the secondary bound: ~9ns+bytes/27GBps per
    desc, serial per engine) have to carry twice.
  - single_packet=True (<=57 descs/ring/call, CH=896) is load-bearing:
    single_packet=False degrades to per-descriptor ring packets (~32
    vs ~14-18ns/desc at the DMA engine).
  - PREPARE_ONLY + batched lagged triggers: a gen_mode=0 call retires
    only when its DMA completes, so 4-deep rounds ran at call-lifetime
    cadence. Preps retire at emission end; triggers are batched 4 at a
    time, lagged 8 calls, so their prep-sem waits resolve while the
    pipeline is still draining earlier preps (sem-prop hidden). Ring
    capacity check: 2 calls x 57 descs = 114 <= 128 slots/ring/queue.
  - Warm idx from a DVE memset (no input dependency); the warmup call
    absorbs the ~10us cold Q7/ucode library init that follows
    load_library before the first dma_gather can execute.
  - fp16 table/output halve both HBM directions (rel err 2^-11 vs the
    2e-2 gate); per-bin output writes overlap the gathers on the two
    HWDGE rings; the last bin's final (tiny) chunk gets its own sem so
    the tail write waits only on it.

dma_gather layout (probed on HW): indices are int16, wrapped over 16
partitions (ordinal i reads idx[i % 16, i // 16]) and replicated to all
8 Q7-core partition groups; gathered ordinal i lands at
dst[i % 128, i // 128]; negative idxs at the end generate no
descriptors, but each call keeps >= 16 leading non-negative idxs so all
16 engine rings still fire their completion-semaphore descriptor.
"""

import time

import numpy as np

VOCAB = 1_000_000
BATCH = 8192
SLOTS = 26
VEC = 128
NCORES = 8
NSUB = 4  # bins per core; int16 gather idx needs rows <= 32767
RSUB = VOCAB // (NCORES * NSUB)  # 31250 rows per bin
SHARD = RSUB * NSUB  # 125000 rows per core
P = 128
CH = 896  # idxs per call: 56 data + 1 sem desc per ring, <= 64 ceiling
NCLS = 2  # class 0: single rows (256B descs); class 1: adjacent pairs (512B)
ROWS_PER = [1, 2]
TRIG_LAG = 8  # preps to run ahead of triggers (2 calls/queue in ring)

LAST_RUN = None  # BassKernelResults of the most recent device run (for test.py)


def _chunks_of(N: int):
    out = []
    o = 0
    while o < N:
        out.append((o, min(CH, N - o)))
        o += CH
    return out


def _build_program(NCL: list, chunks: list):
    """One SPMD program for all 8 cores. NCL[c] = padded idx slots for
    class c per bin (multiples of 128, identical across cores/bins).

    Per core:
      shard [SHARD, VEC] fp16   - this core's 4 bins, concatenated
      idx   [P, ICOLS] i16      - [bin0 c0,c1][bin1 c0,c1]...
      cnt   [1, NCALL] i32      - per-gather-call runtime num_idxs
      out   [P, NSUB*W] fp16    - W = NCL[0] + 2*NCL[1] cols per bin
    """
    import bass_rust
    import concourse.bacc as bacc
    from concourse import mybir
    from concourse.library_config import mlp

    ncalls_bin = sum(len(ch) for ch in chunks)
    icols_bin = sum(NCL) // 16
    ccols = [NCL[c] * ROWS_PER[c] for c in range(NCLS)]
    roff = [0, ccols[0]]
    W = sum(ccols)
    ICOLS = NSUB * icols_bin
    NCALL = NSUB * ncalls_bin

    nc = bacc.Bacc("TRN2", num_swdge_queues=4)
    shard = nc.declare_dram_parameter(
        "shard", [SHARD, VEC], mybir.dt.float16, isOutput=False
    )
    idx = nc.declare_dram_parameter("idx", [P, ICOLS], mybir.dt.int16, isOutput=False)
    cnt = nc.declare_dram_parameter("cnt", [1, NCALL], mybir.dt.int32, isOutput=False)
    out = nc.declare_dram_parameter(
        "out", [P, NSUB * W], mybir.dt.float16, isOutput=True
    )

    sem_in = nc.alloc_semaphore("sem_in")
    sem_warm = nc.alloc_semaphore("sem_warm")
    sem_wi = nc.alloc_semaphore("sem_wi")
    sem_prep = nc.alloc_semaphore("sem_prep")
    # per-QUEUE completion sems: SWDGE completions are FIFO within a
    # queue, so write k of queue q can wait on an exact sem_q[q] count.
    sem_q = [nc.alloc_semaphore(f"sem_q{i}") for i in range(4)]
    sem_out = nc.alloc_semaphore()

    idx_sb = nc.alloc_sbuf_tensor("idx_sb", [P, ICOLS], mybir.dt.int16).ap()
    warm_idx = nc.alloc_sbuf_tensor("warm_idx", [P, 8], mybir.dt.int16).ap()
    cnt_sb = nc.alloc_sbuf_tensor("cnt_sb", [1, NCALL], mybir.dt.int32).ap()
    warm_out = nc.alloc_sbuf_tensor("warm_out", [P, 1, VEC], mybir.dt.float16).ap()
    g_buf = nc.alloc_sbuf_tensor("g", [P, NSUB * W], mybir.dt.float16).ap()

    nc.gpsimd.load_library(mlp)
    nc.vector.memset(warm_idx, 0).then_inc(sem_wi, 1)
    nc.sync.dma_start(out=cnt_sb[:], in_=cnt[:, :]).then_inc(sem_in, 16)
    for s in range(NSUB):
        a, b = s * icols_bin, (s + 1) * icols_bin
        nc.sync.dma_start(out=idx_sb[:, a:b], in_=idx[:, a:b]).then_inc(sem_in, 16)

    warm_reg = nc.gpsimd.to_reg(128)
    cregs = [nc.gpsimd.alloc_register(name=f"creg{t}") for t in range(NCALL)]

    nc.gpsimd.wait_ge(sem_wi, 1)
    nc.gpsimd.dma_gather(
        warm_out[:, :, :],
        shard[0:RSUB, :],
        warm_idx,
        128,
        warm_reg,
        VEC,
        queue_num=0,
    ).then_inc(sem_warm, 16)

    # Batched loads, <= 24 regs each (52-wide measured failing to lower).
    nc.gpsimd.wait_ge(sem_in, 16)
    for i in range(0, NCALL, 24):
        j = min(i + 24, NCALL)
        nc.gpsimd.reg_load(cregs[i:j], cnt_sb[0:1, i:j])

    # Call order: rounds of 4 retire together, gated by the slowest
    # member, and the 4th pipeline slot runs ~25% slower than the other
    # three (measured). So arrange calls so that every 4th emitted call
    # is a small (tail) chunk: full 896-desc calls in slots 1-3, ragged
    # tails in slot 4. All bins' idx are waited for upfront (they land
    # by ~10us, before the ~16us post-library init anyway).
    order = call_order(NCL, chunks)
    assert len(order) == NCALL

    # cnt values follow emission order; the caller builds cnt in the
    # same (s, c, o) order via call_order().
    nc.gpsimd.wait_ge(sem_in, 16 * (NSUB + 1))
    t = 0
    trig_q = []  # queue of call t (FIFO); triggers batched 4, lagged TRIG_LAG
    ntrig = 0

    def fire_triggers(upto):
        nonlocal ntrig
        if upto > ntrig:
            nc.gpsimd.wait_ge(sem_prep, upto)
            while ntrig < upto:
                nc.gpsimd.trigger_dma(count=1, queue_num=trig_q[ntrig])
                ntrig += 1

    views = {}
    for s in range(NSUB):
        views[(s, 0)] = shard[s * RSUB : (s + 1) * RSUB, :]
        L = ROWS_PER[1]
        v = shard[s * RSUB : s * RSUB + (RSUB - L + 1), :].copy()
        v.ap = bass_rust.VecI64Pair([[VEC, RSUB - L + 1], [1, L * VEC]])
        views[(s, 1)] = v

    wcols = []  # (queue, ordinal_in_queue, col0, col1) per call
    qpos = [0, 0, 0, 0]
    for s, c, o, sz in order:
        L = ROWS_PER[c]
        q = (t + 1) % 4  # warmup used q0; first call on q1
        ibase = s * icols_bin + sum(NCL[:c]) // 16
        c0 = s * W + roff[c] + (o // 128) * L * VEC
        c1 = s * W + roff[c] + ((o + sz) // 128) * L * VEC
        dst = g_buf[:, c0:c1].rearrange("p (k e) -> p k e", e=L * VEC)
        nc.gpsimd.dma_gather(
            dst,
            views[(s, c)],
            idx_sb[:, ibase + o // 16 : ibase + (o + sz) // 16],
            sz,
            cregs[t],
            L * VEC,
            elem_step=VEC if L > 1 else None,
            prepare_only=True,
            sem=sem_q[q],
            queue_num=q,
        ).then_inc(sem_prep, 1)
        trig_q.append(q)
        qpos[q] += 1
        wcols.append((q, qpos[q], c0, c1))
        t += 1
        if t % 4 == 0 and t >= TRIG_LAG:
            fire_triggers(t - (TRIG_LAG - 4))
    assert t == NCALL
    fire_triggers(NCALL)

    # Per-call chunk writes, alternating between the two HWDGE rings
    # (Sync/Scalar), each waiting on its queue's FIFO completion count.
    # Spreads write traffic through the gather phase and shrinks the
    # tail to the final call's own (small) chunk.
    for i, (q, k, c0, c1) in enumerate(wcols):
        eng = nc.sync if i % 2 == 0 else nc.scalar
        eng.wait_ge(sem_q[q], 16 * k)
        eng.dma_start(out=out[:, c0:c1], in_=g_buf[:, c0:c1]).then_inc(sem_out, 16)
    nc.sync.wait_ge(sem_out, 16 * NCALL)
    nc.sync.wait_ge(sem_warm, 16)
    nc.finalize()
    return nc


def call_order(NCL: list, chunks: list):
    """Emission order of (s, c, o, sz) — must match _build_program."""
    full = []
    tails = []
    for s in range(NSUB):
        for c in range(NCLS):
            for o, sz in chunks[c]:
                (full if sz == CH else tails).append((s, c, o, sz))
    tails.sort(key=lambda x: x[3])
    order = []
    fi, ti = 0, 0
    while fi < len(full) or ti < len(tails):
        for _ in range(3):
            if fi < len(full):
                order.append(full[fi])
                fi += 1
            elif ti < len(tails):
                order.append(tails[ti])
                ti += 1
        if ti < len(tails):
            order.append(tails[ti])
            ti += 1
        elif fi < len(full):
            order.append(full[fi])
            fi += 1
    return order


def _wrap_cols(vals: np.ndarray, N: int, ecount: int) -> np.ndarray:
    """int16 idx block [16, N//16]: element i at [i%16, i//16]; slots
    [len(vals), ecount) hold 0 (valid row, gathered then ignored), slots
    [ecount, N) hold -1 (skipped by the ucode)."""
    li = np.full(N, -1, np.int16)
    li[:ecount] = 0
    li[: len(vals)] = vals.astype(np.int16)
    return li.reshape(N // 16, 16).T


def _split_pairs(rows: np.ndarray):
    """Greedy adjacent pairing of sorted unique rows: returns
    (singles, pair_starts)."""
    n = len(rows)
    if n == 0:
        return rows, rows
    run_start = np.concatenate(([True], np.diff(rows) != 1))
    run_id = np.cumsum(run_start) - 1
    starts = np.flatnonzero(run_start)
    lens = np.diff(np.append(starts, n))
    pos = np.arange(n) - starts[run_id]
    paired = 2 * (lens[run_id] // 2)
    is_pair_start = (pos % 2 == 0) & (pos < paired)
    is_single = pos >= paired
    return rows[is_single], rows[is_pair_start]


def _gather_on_device(table_f16: np.ndarray, uniq: np.ndarray) -> np.ndarray:
    """emb[i] = table[uniq[i]] (fp16) computed on 8 NeuronCores."""
    global LAST_RUN
    from concourse.bass_utils import run_bass_kernel_spmd

    total = uniq.shape[0]
    nbins = NCORES * NSUB
    bin_id = (uniq // RSUB).astype(np.int32)
    local = (uniq - bin_id.astype(np.int64) * RSUB).astype(np.int32)
    counts = np.bincount(bin_id, minlength=nbins)
    assert counts.sum() == total
    bin_start = np.concatenate(([0], np.cumsum(counts)))

    dec = []  # dec[b] = (singles, pair_starts)
    ncls_max = [0, 0]
    for b in range(nbins):
        sgl, prs = _split_pairs(local[bin_start[b] : bin_start[b + 1]])
        dec.append((sgl, prs))
        ncls_max[0] = max(ncls_max[0], len(sgl))
        ncls_max[1] = max(ncls_max[1], len(prs))
    NCL = [max(P, ((m + P - 1) // P) * P) for m in ncls_max]
    chunks = [_chunks_of(NCL[c]) for c in range(NCLS)]
    ncalls_bin = sum(len(ch) for ch in chunks)
    icols_bin = sum(NCL) // 16
    ccols = [NCL[c] * ROWS_PER[c] for c in range(NCLS)]
    roff = [0, ccols[0]]
    W = sum(ccols)

    order = call_order(NCL, chunks)
    in_maps = []
    for core in range(NCORES):
        blocks = []
        ecounts = {}
        for s in range(NSUB):
            b = core * NSUB + s
            for c in range(NCLS):
                vals = dec[b][c]
                n = len(vals)
                o_last = chunks[c][-1][0]
                ecount = max(n, o_last + 16)
                ecounts[(s, c)] = ecount
                blocks.append(_wrap_cols(vals, NCL[c], ecount))
        cvals = [
            min(max(ecounts[(s, c)] - o, 0), sz) for s, c, o, sz in order
        ]
        in_maps.append(
            {
                "shard": np.ascontiguousarray(
                    table_f16[core * SHARD : (core + 1) * SHARD]
                ),
                "idx": np.ascontiguousarray(
                    np.tile(np.concatenate(blocks, axis=1), (8, 1))
                ),
                "cnt": np.array([cvals], np.int32),
            }
        )

    # The shared device occasionally wedges transiently
    # (NRT_EXEC_UNIT_UNRECOVERABLE / profile-stop rc=-1); a fresh attempt
    # after a short pause recovers it.
    for attempt in range(3):
        try:
            nc = _build_program(NCL, chunks)
            LAST_RUN = run_bass_kernel_spmd(nc, in_maps, list(range(NCORES)))
            break
        except Exception:
            if attempt == 2:
                raise
            time.sleep(10)
    res = LAST_RUN.results

    emb = np.empty((total, VEC), np.float16)
    for core in range(NCORES):
        o = np.asarray(res[core]["out"])
        for s in range(NSUB):
            b = core * NSUB + s
            sgl, prs = dec[b]
            reg = o[:, s * W : (s + 1) * W]
            bs = bin_start[b]
            loc = local[bs : bin_start[b + 1]]
            # class 0: singles; ordinal i at [i%128, (i//128)*VEC + :]
            if len(sgl):
                r0 = reg[:, roff[0] : roff[0] + ccols[0]]
                rows = (
                    r0.reshape(P, NCL[0] // 128, VEC).transpose(1, 0, 2).reshape(-1, VEC)
                )
                pos = np.searchsorted(loc, sgl)
                emb[bs + pos] = rows[: len(sgl)]
            # class 1: pairs; ordinal j covers rows (p_j, p_j + 1)
            if len(prs):
                r1 = reg[:, roff[1] : roff[1] + ccols[1]]
                pairs = (
                    r1.reshape(P, NCL[1] // 128, 2 * VEC)
                    .transpose(1, 0, 2)
                    .reshape(-1, 2, VEC)[: len(prs)]
                )
                pos0 = np.searchsorted(loc, prs)
                emb[bs + pos0] = pairs[:, 0]
                emb[bs + pos0 + 1] = pairs[:, 1]
    return emb


def kernel(table, row_offsets, value_tensors, nnz_array=None, output_shape=None):
    table = np.asarray(table, dtype=np.float32)
    assert table.shape == (VOCAB, VEC)
    v = np.asarray(value_tensors).astype(np.int64).ravel()
    total = v.shape[0]

    table_f16 = table.astype(np.float16)
    uniq, inverse = np.unique(v, return_inverse=True)
    emb_u = _gather_on_device(table_f16, uniq)
    emb = emb_u[inverse].astype(np.float32)

    n_rows = BATCH * SLOTS
    ro = np.asarray(row_offsets).astype(np.int64).ravel()
    if total == n_rows and np.array_equal(ro, np.arange(total + 1)):
        return emb.reshape(BATCH, SLOTS, VEC)
    # General CSR fallback (never hit with the reference's arange offsets):
    # sum-combine values per segment on the host.
    seg = np.searchsorted(ro, np.arange(total), side="right") - 1
    combined = np.zeros((n_rows, VEC), np.float32)
    np.add.at(combined, seg, emb)
    return combined.reshape(BATCH, SLOTS, VEC)
